# revision 36
# baseline (speedup 1.0000x reference)
"""Trainium2 Bass kernel for HeadTailBoundaryPredictor.

Reference computation (B=8, S=512, E=16, H=768):
    t   = token @ Wt.T + bt                    [B,S,H]
    e2  = ent @ We.T + be                      [B,E,H]
    cls = einsum('besh,h->bes', relu(t[:,None]+e2[:,:,None]), wb)
    cls = where(mask, cls, -1e4); p = sigmoid(cls)

Math restructure 1: fold |wb| into the projections. With a = |wb|, s = sign(wb):
    cls[e,s] = sum_o s[o] * relu( zu[s,o] + zv[e,o] )
where zu = a*(Wt tok), zv = a*(We ent) + a*(bt+be), since a*relu(x) = relu(a*x).

Math restructure 2 (the big one): relu(zu+zv) = max(zu, -zv) + zv, so
    cls[e,s] = sum_o s[o]*max(zu[s,o], -zv[e,o])  +  D[e],
    D[e] = sum_o s[o]*zv[e,o].
max(zu, -zv) is a pure two-tensor elementwise op -> ONE DVE tensor_tensor(max)
covers many entities per instruction (broadcast APs), instead of one
tensor_scalar per entity. D[e] is a cheap rank-1 reduction; it is applied as a
per-partition bias in the tail (cls copy + sigmoid both take bias APs).

Device plan (per core = one batch, data-parallel over B):
  - Host compacts the sequence dim: only token positions with mask=1 are
    shipped/computed (S_c = roundup(max_count, 32)); masked outputs are the
    constants -1e4 / sigmoid(-1e4)=0, filled host-side.
  - zu'T[o,s] per o-chunk j: bf16 matmuls (TensorE), PSUM -> f16 u_sb (ScalarE)
  - zv'T[o,e]: bf16 matmuls into ps_v
  - act tiles:
      entities 0..11 (quads 0..2): DVE tensor_tensor(max) in 2 instructions
        per chunk (6 entities each) via broadcast APs: in0 = u_sb broadcast
        over e, in1 = -zv replicated 4x along an inner dim so the last AP dim
        is packed (2x_1p DVE mode needs stride-1 last dim).
      entities 12..15 (quad 3): ScalarE activation(Relu, bias=zv) from PSUM
        directly (classic form, no D correction for quad 3).
  - cls[e,s] = sgnT @ act: 1-column f16 matmuls rotated over the 4 PE column
    groups (tile_position), PSUM-accumulated over o-chunks.
  - D: per chunk one 12-column matmul (sgn_j stationary, zv f16 moving) into
    ps_d; host-known constant sum(sgn*a*(bt+be)) added on the SBUF copy; a
    16-descriptor SBUF->SBUF DMA scatters D[e] to partition 32g col q.
  - tail per quad: DVE cls copy (bias +D), ScalarE sigmoid (bias +D), one DMA.

Schedule notes:
  - PE warms up on a memset scratch tile from t=0 (no DMA dependency) so the
    p-state is ramped when the first real matmul issues.
  - DMA order: tok halves on SP+ACT queues first, Wt slices on SP in
    consumption order, ent/We0/bb/sgn/We1 on ACT, We2..5 on the GpSimd SWDGE.
"""

import sys

for _p in ("/opt/trn_rl_repo", "/root/.axon_site/_ro/trn_rl_repo"):
    if _p not in sys.path:
        sys.path.append(_p)

import numpy as np
import ml_dtypes

import concourse.bass as bass
import concourse.mybir as mybir
import concourse.tile as tile
from concourse.bass_utils import run_bass_kernel_spmd

dt = mybir.dt
AF = mybir.ActivationFunctionType
ALU = mybir.AluOpType

B, S, E, H = 8, 512, 16, 768
P = 128
NH = H // P  # 6 chunks of the hidden/output dims
NQ = E // 4  # 4 entity quads (one PSUM bank each)

N_WARMUP = 40
ND = 12  # entities 0..11 on DVE (quads 0..2, D-corrected); 12..15 on ScalarE
ACT_ENTS = [12, 13, 14, 15]
REDUCE_ORDER = ACT_ENTS + list(range(ND))

_WAITSPLIT_CTR = [0]


def _split_excess_waits(nc, limit=1):
    """walrus (CoreV3) accepts at most `limit` sync-wait commands per
    instruction; Tile can emit more (e.g. the tail drain). Move excess waits
    onto freshly inserted same-engine NoOps, which is semantically identical."""
    n = 0
    for f in nc.m.functions:
        for bb in f.blocks:
            insts = list(bb.instructions)
            out = []
            changed = False
            for inst in insts:
                si = inst.sync_info
                waits = list(si.on_wait) if si else []
                if len(waits) > limit:
                    head, tail = waits[:-limit], waits[-limit:]
                    for i in range(0, len(head), limit):
                        _WAITSPLIT_CTR[0] += 1
                        nop = mybir.InstNoOp(
                            name=f"waitsplit_nop_{_WAITSPLIT_CTR[0]}", ins=[], outs=[]
                        )
                        nop.engine = inst.engine
                        nop.sync_info = mybir.SyncInfo(
                            on_wait=head[i : i + limit], on_update=[]
                        )
                        out.append(nop)
                        n += 1
                    si.on_wait = tail
                    inst.sync_info = si
                    changed = True
                out.append(inst)
            if changed:
                bb.instructions = out
    return n


def _tensor_tensor(eng, out, in0, in1, op):
    """Plain element-wise tensor-tensor op (bass has scan/reduce variants but
    no bare emitter). Supports DVE 2x_1p when all APs are 2-byte packed."""
    return eng.add_instruction(
        mybir.InstTensorTensor(
            name=eng.bass.get_next_instruction_name(),
            op=op,
            ins=[eng.lower_ap(in0), eng.lower_ap(in1)],
            outs=[eng.lower_ap(out)],
        )
    )


def _build_nc(S_c, d0):
    SQ = S_c // 4
    nc = bass.Bass()

    tok_pk = nc.dram_tensor("tok_pk", [P, NH * S_c], dt.bfloat16, kind="ExternalInput")
    wt_pk = nc.dram_tensor("wt_pk", [P, NH * NH * P], dt.bfloat16, kind="ExternalInput")
    we_pk = nc.dram_tensor("we_pk", [P, NH * NH * P], dt.bfloat16, kind="ExternalInput")
    # aux = [ent (96) | sgn (6) | bb (6)] all bf16, one small DMA
    aux_pk = nc.dram_tensor("aux_pk", [P, NH * E + 2 * NH], dt.bfloat16, kind="ExternalInput")

    # out[q, g, c]: entity e = 4q+g; c = [cls | p] each S_c wide
    out_t = nc.dram_tensor("out", [NQ, 4, 2 * S_c], dt.float16, kind="ExternalOutput")

    with tile.TileContext(nc) as tc:
        with (
            tc.tile_pool(name="const", bufs=1) as cpool,
            tc.tile_pool(name="wts", bufs=1) as wpool,
            tc.tile_pool(name="usb", bufs=3) as upool,
            tc.tile_pool(name="acts", bufs=3) as apool,
            tc.tile_pool(name="sacts", bufs=12) as sapool,
            tc.tile_pool(name="outs", bufs=1) as opool,
            tc.tile_pool(name="psv", bufs=1, space="PSUM") as psv,
            tc.tile_pool(name="psu", bufs=3, space="PSUM") as psu,
            tc.tile_pool(name="psc", bufs=1, space="PSUM") as psc,
        ):
            # ---- SBUF tiles ----
            t_aux = cpool.tile([P, NH * E + 2 * NH], dt.bfloat16, tag="aux")
            t_sgn = cpool.tile([P, NH], dt.float16, tag="sgn")
            t_bb = cpool.tile([P, NH], dt.float32, tag="bb")
            t_wscr = cpool.tile([P, 256], dt.float16, tag="wscr")
            v_sb = cpool.tile([P, NH * 4], dt.float32, tag="vsb")  # ACT ents
            nv4 = cpool.tile([P, NH * ND * 4], dt.float16, tag="nv4")  # -zv rep4
            t_ones = cpool.tile([P, 1], dt.float16, tag="ones")
            sgn32 = cpool.tile([P, NH], dt.float32, tag="sgn32")
            sgz = [None] * NH  # running sum_j sgn_j * zv_j  [P, ND]
            d_colq = cpool.tile([P, NQ], dt.float32, tag="dcol")
            t_wt = wpool.tile([P, NH * NH * P], dt.bfloat16, tag="wt")
            t_we = wpool.tile([P, NH * NH * P], dt.bfloat16, tag="we")
            t_tok = wpool.tile([P, NH * S_c], dt.bfloat16, tag="tok")
            osb = opool.tile([P, NQ * 2 * S_c], dt.float16, tag="osb")

            # ---- DMA issue ----
            # GpSimd: first SWDGE DMA ahead of the memsets so its ~1us
            # descriptor gen starts immediately; memsets are cheap
            nc.gpsimd.dma_start(t_tok[:, (NH // 2) * S_c :], tok_pk[:, (NH // 2) * S_c :])
            nc.gpsimd.memset(t_wscr[:], 0.0)
            nc.gpsimd.memset(t_ones[:], 1.0)
            nc.gpsimd.memset(d_colq[:], 0.0)
            # Every DMA costs ~1.3-1.6us of queue overhead regardless of
            # size (measured), so: few, large, need-ordered transfers.
            wsl = lambda j: slice(j * NH * P, (j + 1) * NH * P)
            half = (NH // 2) * S_c
            # SP queue: token front half + wt0 first, then mid Wt slices
            nc.sync.dma_start(t_tok[:, :half], tok_pk[:, :half])
            nc.sync.dma_start(t_wt[:, wsl(0)], wt_pk[:, wsl(0)])
            nc.sync.dma_start(t_wt[:, wsl(2)], wt_pk[:, wsl(2)])
            nc.sync.dma_start(t_wt[:, wsl(3)], wt_pk[:, wsl(3)])
            # ACT queue: aux smalls first, then wt1/wt4
            nc.scalar.dma_start(t_aux[:], aux_pk[:])
            nc.scalar.dma_start(t_wt[:, wsl(1)], wt_pk[:, wsl(1)])
            nc.scalar.dma_start(t_wt[:, wsl(4)], wt_pk[:, wsl(4)])
            # GpSimd SWDGE: We slices in consumption order (tok back half
            # was issued before the memsets above)
            nc.gpsimd.dma_start(t_we[:, wsl(0)], we_pk[:, wsl(0)])
            nc.gpsimd.dma_start(t_we[:, wsl(1)], we_pk[:, wsl(1)])
            nc.gpsimd.dma_start(t_wt[:, wsl(5)], wt_pk[:, wsl(5)])
            nc.gpsimd.dma_start(t_we[:, wsl(2)], we_pk[:, wsl(2)])
            nc.gpsimd.dma_start(t_we[:, wsl(3)], we_pk[:, wsl(3)])
            nc.gpsimd.dma_start(t_we[:, 4 * NH * P :], we_pk[:, 4 * NH * P :])

            # unpack aux: ent stays bf16 in place; sgn -> f16 (reduce
            # stationary) and f32 (DVE mult scalars); bb -> f32
            t_ent = t_aux[:, 0 : NH * E]
            nc.vector.tensor_scalar(
                t_sgn[:], t_aux[:, NH * E : NH * E + NH], 0.0, None, op0=ALU.add
            )
            nc.vector.tensor_scalar(
                t_bb[:], t_aux[:, NH * E + NH : NH * E + 2 * NH], 0.0, None, op0=ALU.add
            )

            # f32 copy of sgn for the DVE mult scalars (mult wants f32)
            nc.vector.tensor_scalar(sgn32[:], t_sgn[:], 0.0, None, op0=ALU.add)

            # dummy sigmoid pulls the ACT table load off the critical path
            t_dmy = cpool.tile([P, 1], dt.float32, tag="dmy")
            nc.scalar.activation(t_dmy[:], t_wscr[:, 0:1], AF.Sigmoid)

            # ---- PSUM tiles ----
            ps_c = [
                psc.tile([P, S_c], dt.float32, tag=f"cq{q}", name=f"ps_c{q}")
                for q in range(NQ)
            ]
            ps_vd = psv.tile([P, NH * E + NQ], dt.float32, tag="vps")
            ps_v = ps_vd[:, 0 : NH * E]
            ps_dg = ps_vd[:, NH * E : NH * E + NQ]

            # ---- PE warmup (p-state ramp) on the last quad's PSUM bank;
            # no DMA dependency (stationary+moving = memset scratch) ----
            wcols = min(128, S_c)
            for w in range(N_WARMUP):
                nc.tensor.matmul(
                    ps_c[NQ - 1][0:1, 0:wcols],
                    t_wscr[:, 0:1],
                    t_wscr[:, 0:wcols],
                    start=True,
                    stop=True,
                )

            u_sb = [None] * NH
            ps_u_t = [None] * NH
            act_d = [None] * NH  # DVE act tiles [P, ND*S_c]
            act_s = [[None] * 4 for _ in range(NH)]  # ScalarE act tiles

            def uproj(j):
                ps_u = psu.tile([P, S_c], dt.float32, tag="ups", name=f"ps_u{j}")
                ps_u_t[j] = ps_u
                for k in range(NH):
                    nc.tensor.matmul(
                        ps_u[:],
                        t_wt[:, (j * NH + k) * P : (j * NH + k + 1) * P],
                        t_tok[:, k * S_c : (k + 1) * S_c],
                        start=(k == 0),
                        stop=(k == NH - 1),
                    )
                # f16 copy for the DVE act tiles (2x mode needs SBUF f16)
                u_sb[j] = upool.tile([P, S_c], dt.float16, tag="u", name=f"u_sb{j}")
                nc.scalar.copy(u_sb[j][:], ps_u[:])

            def vproj(j, mms=True):
                if mms:
                    for k in range(NH):
                        vproj_mm(j, k)
                bbj = t_bb[:, j : j + 1]
                # -zv replicated 4x (f16) for the DVE tensor_tensor(max)
                src = ps_v[:, j * E : j * E + ND].unsqueeze(2).broadcast_to([P, ND, 4])
                dst = nv4[:, j * ND * 4 : (j + 1) * ND * 4].rearrange(
                    "p (e r) -> p e r", r=4
                )
                nc.vector.tensor_scalar(dst, src, bbj, -1.0, op0=ALU.add, op1=ALU.mult)
                # running sum_j sgn_j*zv_j (bb excluded; host const d0 covers it)
                sgz[j] = cpool.tile([P, ND], dt.float16, tag=f"sgz{j}", name=f"sgz{j}")
                sgnj = sgn32[:, j : j + 1]
                if j == 0:
                    nc.vector.tensor_scalar(
                        sgz[0][:], ps_v[:, 0:ND], sgnj, None, op0=ALU.mult
                    )
                else:
                    nc.vector.scalar_tensor_tensor(
                        sgz[j][:],
                        ps_v[:, j * E : j * E + ND],
                        sgnj,
                        sgz[j - 1][:],
                        op0=ALU.mult,
                        op1=ALU.add,
                    )
                # f32 bias columns for the ScalarE entities
                nc.vector.tensor_scalar(
                    v_sb[:, j * 4 : j * 4 + (E - ND)],
                    ps_v[:, j * E + ND : (j + 1) * E],
                    bbj,
                    None,
                    op0=ALU.add,
                )

            def acts(j, splits):
                # ScalarE entities read PSUM directly (no u-copy dep)
                for g in range(E - ND):
                    a = sapool.tile([P, S_c], dt.float16, tag="sact", name=f"sa_{j}_{g}")
                    act_s[j][g] = a
                    bias = v_sb[:, j * 4 + g : j * 4 + g + 1]
                    nc.scalar.activation(a[:], ps_u_t[j][:], AF.Relu, bias=bias)
                # DVE: max(zu, -zv), several entities per instruction
                act_d[j] = apool.tile([P, ND * S_c], dt.float16, tag="dact", name=f"da_{j}")
                for e0, ne in splits:
                    u4 = (
                        u_sb[j][:]
                        .rearrange("p (a r) -> p a r", r=4)
                        .unsqueeze(1)
                        .broadcast_to([P, ne, SQ, 4])
                    )
                    in1 = nv4[
                        :, (j * ND + e0) * 4 : (j * ND + e0 + ne) * 4
                    ].rearrange("p (e r) -> p e r", r=4).unsqueeze(2).broadcast_to(
                        [P, ne, SQ, 4]
                    )
                    outap = act_d[j][
                        :, e0 * S_c : (e0 + ne) * S_c
                    ].rearrange("p (e a r) -> p e a r", a=SQ, r=4)
                    _tensor_tensor(nc.vector, outap, u4, in1, ALU.max)

            def vproj_mm(jv, k):
                nc.tensor.matmul(
                    ps_v[:, jv * E : (jv + 1) * E],
                    t_we[:, (jv * NH + k) * P : (jv * NH + k + 1) * P],
                    t_ent[:, k * E : (k + 1) * E],
                    start=(k == 0),
                    stop=(k == NH - 1),
                )

            def reduce(j, jv=None):
                # jv: vproj chunk whose (LDW, tiny-mm) pairs hide in the
                # shadows of the 283ns reduce matmuls
                for i, e in enumerate(REDUCE_ORDER):
                    q, g = e // 4, e % 4
                    mov = (
                        act_s[j][e - ND][:]
                        if e >= ND
                        else act_d[j][:, e * S_c : (e + 1) * S_c]
                    )
                    nc.tensor.matmul(
                        ps_c[q][32 * g : 32 * g + 1, :],
                        t_sgn[:, j : j + 1],
                        mov,
                        start=(j == 0),
                        stop=(j == NH - 1),
                        tile_position=(0, 32 * g),
                    )
                    if jv is not None and i >= 2 and i % 2 == 0 and (i - 2) // 2 < NH:
                        vproj_mm(jv, (i - 2) // 2)

            # PE program order: per-j u/v interleaved with reduce lagging one
            # chunk so act tiles are ready when their reduce matmuls issue.
            SPL = [(0, 4), (4, 4), (8, 4)]
            SPL_LAST = SPL
            uproj(0)
            vproj(0)
            acts(0, SPL)
            uproj(1)
            vproj(1)
            acts(1, SPL)
            reduce(0)
            for j in range(2, NH):
                uproj(j)
                vproj(j)
                acts(j, SPL if j < NH - 1 else SPL_LAST)
                reduce(j - 1)

            # D: four 3-col matmuls (ones stationary) land D[4q+g] at
            # partition 32g, col q of ps_dg; one DVE copy (+host const d0)
            # turns that into the SBUF bias columns the tail ops need.
            for g in range(4):
                ncols = len(range(g, ND, 4))
                nc.tensor.matmul(
                    ps_dg[32 * g : 32 * g + 1, 0:ncols],
                    t_ones[:],
                    sgz[NH - 1][:, g:ND:4],
                    start=True,
                    stop=True,
                    tile_position=(0, 32 * g),
                )
            nc.vector.tensor_scalar(
                d_colq[:, 0:3], ps_dg[:, 0:3], d0, None, op0=ALU.add
            )


            # last chunk: finish one quad at a time; its tail (cls copy on DVE,
            # sigmoid on ACT, output DMA) starts while later quads still reduce.
            # Quad 3 first: its entities are ScalarE-owned and need no D.
            j = NH - 1
            for q in range(NQ):
                for g in range(4):
                    e = 4 * q + g
                    mov = (
                        act_s[j][e - ND][:]
                        if e >= ND
                        else act_d[j][:, e * S_c : (e + 1) * S_c]
                    )
                    nc.tensor.matmul(
                        ps_c[q][32 * g : 32 * g + 1, :],
                        t_sgn[:, j : j + 1],
                        mov,
                        start=False,
                        stop=True,
                        tile_position=(0, 32 * g),
                    )
                dbias = 0.0 if q == 3 else d_colq[:, q : q + 1]
                nc.vector.tensor_scalar(
                    osb[:, q * 2 * S_c : q * 2 * S_c + S_c],
                    ps_c[q][:],
                    dbias,
                    None,
                    op0=ALU.add,
                )
                nc.scalar.activation(
                    osb[:, q * 2 * S_c + S_c : (q + 1) * 2 * S_c],
                    ps_c[q][:],
                    AF.Sigmoid,
                    bias=dbias,
                )
                if q in (1, 3):
                    # one DMA per quad-pair (q-1, q); per-DMA overhead is
                    # ~1.6us so fewer transfers beat earlier issue
                    oeng = nc.sync if q == 1 else nc.scalar
                    oeng.dma_start(
                        out_t[q - 1 : q + 1, :, :].rearrange("q g c -> g q c"),
                        osb[0 : P : 32, (q - 1) * 2 * S_c : (q + 1) * 2 * S_c].rearrange(
                            "p (q c) -> p q c", q=2
                        ),
                    )

    _split_excess_waits(nc, limit=1)
    return nc


_NC_CACHE = {}


def _get_nc(S_c, d0):
    key = (S_c, round(float(d0), 6))
    if key not in _NC_CACHE:
        _NC_CACHE[key] = _build_nc(S_c, float(d0))
    return _NC_CACHE[key]


def _pack_pmajor(mat, ncols):
    """[H, ncols] -> [P, NH*ncols] partition-major: out[p, k*ncols+c] =
    mat[k*P+p, c]."""
    return np.ascontiguousarray(
        mat.reshape(NH, P, ncols).transpose(1, 0, 2).reshape(P, NH * ncols)
    )


def kernel(token_embedding, entity_embedding, token_mask, Wt, bt, We, be, wb, **kw):
    token_embedding = np.asarray(token_embedding, dtype=np.float32)
    entity_embedding = np.asarray(entity_embedding, dtype=np.float32)
    token_mask = np.asarray(token_mask).astype(bool)
    Wt = np.asarray(Wt, dtype=np.float32)
    bt = np.asarray(bt, dtype=np.float32)
    We = np.asarray(We, dtype=np.float32)
    be = np.asarray(be, dtype=np.float32)
    wb = np.asarray(wb, dtype=np.float32)

    bf16 = ml_dtypes.bfloat16

    a = np.abs(wb)
    sgn = np.where(wb >= 0, np.float32(1.0), np.float32(-1.0))

    # fold |wb| into the weights; transpose to [h, o]
    W2t = (Wt * a[:, None]).T.astype(np.float32)  # [h, o]
    W2e = (We * a[:, None]).T.astype(np.float32)
    bb = ((bt + be) * a).astype(np.float32)
    d0 = float(np.dot(sgn, bb))  # sum_o sgn*a*(bt+be), folded into D on device

    # wt_pk[p, (j*NH+k)*P + c] = W2[k*P+p, j*P+c]  (j-major blocks)
    def pack_w(W2):
        arr = W2.reshape(NH, P, NH, P).transpose(1, 2, 0, 3)  # [p, j, k, c]
        return np.ascontiguousarray(arr.reshape(P, NH * NH * P)).astype(bf16)

    wt_pk = pack_w(W2t)
    we_pk = pack_w(W2e)
    sgn_pk = np.ascontiguousarray(sgn.reshape(NH, P).T).astype(bf16)
    bb_pk = np.ascontiguousarray(bb.reshape(NH, P).T).astype(bf16)

    idxs = [np.nonzero(token_mask[b])[0] for b in range(B)]
    nmax = max((len(ix) for ix in idxs), default=1)
    S_c = max(64, -(-nmax // 32) * 32)

    nc = _get_nc(S_c, d0)
    in_maps = []
    for b in range(B):
        ix = idxs[b]
        tokc = np.zeros((S_c, H), dtype=np.float32)
        tokc[: len(ix)] = token_embedding[b][ix]
        tok_pk = _pack_pmajor(tokc.T, S_c).astype(bf16)  # [P, NH*S_c]
        ent_pk = _pack_pmajor(entity_embedding[b].T, E).astype(bf16)
        aux_pk = np.concatenate([ent_pk, sgn_pk, bb_pk], axis=1)
        in_maps.append(
            {
                "tok_pk": tok_pk,
                "wt_pk": wt_pk,
                "we_pk": we_pk,
                "aux_pk": np.ascontiguousarray(aux_pk),
            }
        )

    res = run_bass_kernel_spmd(nc, in_maps, core_ids=list(range(B)))

    cls = np.full((B, E, S), -10000.0, dtype=np.float32)
    p = np.zeros((B, E, S), dtype=np.float32)
    for b in range(B):
        o = np.asarray(res.results[b]["out"], dtype=np.float32).reshape(E, 2 * S_c)
        ix = idxs[b]
        cls[b][:, ix] = o[:, : len(ix)]
        p[b][:, ix] = o[:, S_c : S_c + len(ix)]
    return cls, p
